# revision 1
# baseline (speedup 1.0000x reference)
import numpy as np
import jax
import jax.numpy as jnp

# nn_DAGLSTM: B=16, N=128, E=1024, D=256, L=2, NCLS=7.
# Sharding: pure data parallelism over batch B across 8 cores (2 per core);
# the DAG recurrence over utterances is strictly sequential, so each core
# runs the full recurrence for its batch shard; small weights replicated.

L = 2
D = 256
E = 1024
NCLS = 7
NEG = 1e30
B, N, M = 16, 128, 8

WNAMES = ("fc1_W", "fc1_b", "W1", "W2", "Wr",
          "Wc_ih", "Wc_hh", "bc_ih", "bc_hh",
          "Wp_ih", "Wp_hh", "bp_ih", "bp_hh",
          "m0_W", "m0_b", "m1_W", "m1_b", "m2_W", "m2_b")


def _lstm(x, h, c, Wih, Whh, bih, bhh):
    g = x @ Wih.T + bih + h @ Whh.T + bhh
    i, f, gg, o = jnp.split(g, 4, axis=-1)
    c2 = jax.nn.sigmoid(f) * c + jax.nn.sigmoid(i) * jnp.tanh(gg)
    return jax.nn.sigmoid(o) * jnp.tanh(c2), c2


def _dag_layer(Hl, adj_f, W1, W2, Wr, Wc_ih, Wc_hh, bc_ih, bc_hh,
               Wp_ih, Wp_hh, bp_ih, bp_hh):
    Bs, Nn, Dh = Hl.shape
    zeros = jnp.zeros((Bs, Dh), Hl.dtype)
    h0 = Hl[:, 0]
    C0, _ = _lstm(h0, zeros, zeros, Wc_ih, Wc_hh, bc_ih, bc_hh)
    P0, _ = _lstm(zeros, h0, h0, Wp_ih, Wp_hh, bp_ih, bp_hh)
    Ht0 = C0 + P0
    K2 = jnp.zeros((Bs, Nn, Dh), Hl.dtype).at[:, 0].set(Ht0 @ W2.T)
    Vr = jnp.zeros((Bs, Nn, Dh), Hl.dtype).at[:, 0].set(Ht0 @ Wr.T)
    H1 = jnp.zeros((Bs, Nn, Dh), Hl.dtype).at[:, 0].set(Ht0)
    pos = jnp.arange(Nn)

    def step(carry, i):
        K2, Vr, H1 = carry
        Q = jax.lax.dynamic_index_in_dim(Hl, i, axis=1, keepdims=False)
        adj_i = jax.lax.dynamic_index_in_dim(adj_f, i, axis=1, keepdims=False)
        q = Q @ W1.T
        logits = jnp.einsum('bd,bnd->bn', q, K2) - (1.0 - adj_i) * NEG
        logits = jnp.where(pos[None, :] < i, logits, -NEG)
        w = jax.nn.softmax(logits, axis=-1)
        Mv = jnp.einsum('bn,bnd->bd', w, Vr)
        C, _ = _lstm(Q, Mv, Mv, Wc_ih, Wc_hh, bc_ih, bc_hh)
        P, _ = _lstm(Mv, Q, Q, Wp_ih, Wp_hh, bp_ih, bp_hh)
        Ht = C + P
        K2 = K2.at[:, i].set(Ht @ W2.T)
        Vr = Vr.at[:, i].set(Ht @ Wr.T)
        H1 = H1.at[:, i].set(Ht)
        return (K2, Vr, H1), None

    (K2, Vr, H1), _ = jax.lax.scan(step, (K2, Vr, H1), jnp.arange(1, Nn))
    return H1


def _forward(features, adj, *w):
    p = dict(zip(WNAMES, w))
    adj_f = adj.astype(features.dtype)
    H0 = jax.nn.relu(features @ p["fc1_W"].T + p["fc1_b"])
    Hs = [H0]
    for l in range(L):
        Hs.append(_dag_layer(Hs[l], adj_f,
                             p["W1"][l], p["W2"][l], p["Wr"][l],
                             p["Wc_ih"][l], p["Wc_hh"][l], p["bc_ih"][l], p["bc_hh"][l],
                             p["Wp_ih"][l], p["Wp_hh"][l], p["bp_ih"][l], p["bp_hh"][l]))
    Hcat = jnp.concatenate(Hs + [features], axis=2)
    h = jax.nn.relu(Hcat @ p["m0_W"].T + p["m0_b"])
    h = jax.nn.relu(h @ p["m1_W"].T + p["m1_b"])
    return h @ p["m2_W"].T + p["m2_b"]


_CACHE = {}


def _get_pmapped(n_dev):
    if n_dev not in _CACHE:
        _CACHE[n_dev] = jax.pmap(
            _forward,
            in_axes=(0, 0) + (None,) * len(WNAMES),
            devices=jax.devices()[:n_dev])
    return _CACHE[n_dev]


def kernel(**inputs):
    feats = np.asarray(inputs["features"], np.float32)
    adj = np.asarray(inputs["adj"], np.int32)
    ws = [np.asarray(inputs[k], np.float32) for k in WNAMES]
    n_dev = min(M, len(jax.devices()), B)
    bb = B // n_dev
    fs = feats.reshape(n_dev, bb, N, E)
    asx = adj.reshape(n_dev, bb, N, N)
    try:
        fn = _get_pmapped(n_dev)
        out = fn(fs, asx, *ws)
        out = np.asarray(out).reshape(B, N, NCLS)
    except Exception:
        out = np.asarray(jax.jit(_forward, backend="cpu")(feats, adj, *ws))
    return out.astype(np.float32)

